# revision 10
# baseline (speedup 1.0000x reference)
"""Trainium2 Bass kernel: multi-head causal attention (B=2, T=2048, C=1024, H=16).

Sharding: 8 cores = data parallel over B (2) x tensor parallel over head
groups (4 groups of 4 heads).  Each core computes its batch's partial
output contribution from its 4 heads through Wo rows; the host sums the 4
partials per batch (the "all-reduce") and adds the folded biases.

Device pipeline (per core, 4 heads, matmul operands bf16 / PSUM fp32):
  - Q/K/V arrive bf16 [T, C]; DMA xbar-transpose loads them as [C, T]
    chunks directly into SBUF (no PE transposes). Q/V ride the SP HWDGE
    ring, K + weights + output stores ride the ACT ring (2 parallel rings)
  - qT/kT = W^T @ X^T + b  laid out [head_dim, T]; v kept natural [T, dv]
    with a ones column appended per head (bias bv folded on host)
  - scores computed TRANSPOSED: scT[k_block 128, q 512] = kT_blk^T @ qT
    per (head, q-chunk, k-block); strict-lower-tri -1e9 mask added on the
    diagonal block (DVE); Exp on ACT writes attnT bf16 straight to SBUF
  - av[dv+1, q] += v_aug^T @ attnT accumulated over k blocks in PSUM (one
    contiguous burst per head); the ones column makes row dv the softmax
    denominator S[q] for free
  - normalize: R=1/S (DVE approx-fast), partition_broadcast (Pool),
    av *= Rb -> outT bf16 (DVE)
  - output projection from outT pairs at K=128; fin copies (DVE) as bf16
"""

from contextlib import ExitStack

import numpy as np
import ml_dtypes

import concourse.bass as bass
import concourse.mybir as mybir
import concourse.tile as tile
from concourse import bacc
from concourse.bass_utils import run_bass_kernel_spmd

B, T, C = 2, 2048, 1024
H, DK, DV = 16, 64, 64
N_CORES = 8
GROUPS = 4                 # head groups (tensor parallel)
HPG = H // GROUPS          # 4 heads per group
GD = HPG * DK              # 256 head dims per group
P = 128
TCH = 512                  # chunk of T for wide matmuls / query chunks

BF = mybir.dt.bfloat16
F32 = mybir.dt.float32
AX = mybir.AxisListType
AF = mybir.ActivationFunctionType

bf16 = ml_dtypes.bfloat16

# scheduling knobs; _NC_CACHE keys include these
CFG = {"pipeline": True, "tch_bufs": 3, "attnt_bufs": 2, "sc_bufs": 3,
       "av_bufs": 2, "fin_bufs": 2, "mm_bufs": 2}


def _emit(nc, tc, io, t_len, ctx):
    NT = t_len // P            # key 128-blocks
    NQC = t_len // TCH         # query 512-chunks
    NCB = C // P               # contraction chunks over C

    cpool = ctx.enter_context(tc.tile_pool(name="const", bufs=1))
    spool = ctx.enter_context(tc.tile_pool(name="stream", bufs=2))
    ppool = ctx.enter_context(tc.tile_pool(name="pers", bufs=1))
    apool = ctx.enter_context(tc.tile_pool(name="attn", bufs=2))
    pp = ctx.enter_context(tc.tile_pool(name="ps", bufs=2, space="PSUM"))

    # ---- constants / weights (ACT HWDGE ring) -------------------------------
    amaskT = cpool.tile([P, P], F32)   # strict LOWER triangular -1e9 (k > q)
    nc.scalar.dma_start(out=amaskT, in_=io["amask"][:, :])
    bq_sb = cpool.tile([P, 2], F32)
    nc.scalar.dma_start(out=bq_sb, in_=io["bq"][:, :])
    bk_sb = cpool.tile([P, 2], F32)
    nc.scalar.dma_start(out=bk_sb, in_=io["bk"][:, :])

    wq_sb = cpool.tile([P, NCB, GD], BF)
    wk_sb = cpool.tile([P, NCB, GD], BF)
    wv_sb = cpool.tile([P, NCB, GD], BF)
    for w_sb, name in ((wq_sb, "wq"), (wk_sb, "wk"), (wv_sb, "wv")):
        for cb in range(NCB):
            nc.scalar.dma_start(out=w_sb[:, cb, :], in_=io[name][cb * P:(cb + 1) * P, :])
    wo_sb = cpool.tile([P, 2, C], BF)
    for pr in range(2):
        nc.scalar.dma_start(out=wo_sb[:, pr, :], in_=io["wo"][pr * P:(pr + 1) * P, :])

    # persistent activations
    qT_sb = ppool.tile([P, 2, t_len], BF)   # [pair head dims(128), pair, T]
    kT_sb = ppool.tile([P, 2, t_len], BF)
    # v natural [T(k), head, dv + ones col]; row DV of av becomes sum(exp)
    v_aug = ppool.tile([P, NT, HPG, DV + 2], BF)
    nc.gpsimd.memset(v_aug[:, :, :, DV:DV + 1], 1.0)
    outT_sb = ppool.tile([P, 2, t_len], BF)  # [2 heads' dv, pair, T]

    # ---- stage 1: DMA-transposed loads + projections for one t-chunk --------
    def load_t4(t4):
        t0 = t4 * TCH
        for name, w_sb, bias_sb, xT_sb in (
            ("q", wq_sb, bq_sb, qT_sb),
            ("k", wk_sb, bk_sb, kT_sb),
            ("v", wv_sb, None, None),
        ):
            dma = nc.sync
            tch = spool.tile([P, NCB, TCH], BF, tag="tch", bufs=CFG["tch_bufs"])
            for cb in range(NCB):
                dma.dma_start(
                    out=tch[:, cb, :],
                    in_=io[name][t0:t0 + TCH, cb * P:(cb + 1) * P],
                    transpose=True)
            if name == "v":
                for tb in range(4):
                    ps = pp.tile([P, GD], F32, tag="mm", bufs=CFG["mm_bufs"])
                    for cb in range(NCB):
                        nc.tensor.matmul(
                            ps, tch[:, cb, tb * P:(tb + 1) * P], w_sb[:, cb, :],
                            start=(cb == 0), stop=(cb == NCB - 1))
                    nc.vector.tensor_copy(
                        v_aug[:, t4 * 4 + tb, :, 0:DV],
                        ps.rearrange("p (h d) -> p h d", h=HPG))
            else:
                for pr in range(2):
                    ps = pp.tile([P, TCH], F32, tag="mm", bufs=CFG["mm_bufs"])
                    for cb in range(NCB):
                        nc.tensor.matmul(
                            ps, w_sb[:, cb, pr * P:(pr + 1) * P], tch[:, cb, :],
                            start=(cb == 0), stop=(cb == NCB - 1))
                    nc.vector.tensor_scalar_add(
                        xT_sb[:, pr, t0:t0 + TCH], ps, bias_sb[:, pr:pr + 1])

    # ---- stage 2: attention per (query chunk, head), then project -----------
    def attend_qc(qc):
        nkb = (qc + 1) * 4
        for h in range(HPG):
            pr, half = h // 2, h % 2
            hs = half * DK
            # diagonal blocks first: the d=0 one opens the av accumulation
            # full-width; trailing full blocks close it full-width
            order = list(range(qc * 4, nkb)) + list(range(0, qc * 4))
            attnT = apool.tile([P, nkb, TCH], BF, tag="attnT",
                               bufs=CFG["attnt_bufs"])
            for kb in order:
                d = max(kb * P - qc * TCH, 0)
                sc = pp.tile([P, TCH], F32, tag="sc", bufs=CFG["sc_bufs"])
                nc.tensor.matmul(
                    sc[:, d:], kT_sb[hs:hs + DK, pr, kb * P:(kb + 1) * P],
                    qT_sb[hs:hs + DK, pr, qc * TCH + d:(qc + 1) * TCH],
                    start=True, stop=True)
                if kb * P >= qc * TCH:  # diagonal block: mask k > q
                    nc.vector.tensor_add(sc[:, d:d + P], sc[:, d:d + P], amaskT)
                nc.scalar.activation(
                    attnT[:, kb, d:], sc[:, d:], AF.Exp, scale=0.125)
            av = pp.tile([P, TCH], F32, tag="av", bufs=CFG["av_bufs"])
            for idx, kb in enumerate(order):
                d = max(kb * P - qc * TCH, 0)
                nc.tensor.matmul(
                    av[0:DV + 1, d:], v_aug[:, kb, h, 0:DV + 1],
                    attnT[:, kb, d:],
                    start=(idx == 0), stop=(idx == nkb - 1),
                    skip_group_check=True)
            S_sb = apool.tile([1, TCH], F32, tag="S", bufs=2)
            nc.scalar.copy(S_sb, av[DV:DV + 1, :])
            R32 = apool.tile([1, TCH], F32, tag="R32", bufs=2)
            nc.vector.reciprocal_approx_fast(R32, S_sb)
            Rb = apool.tile([DV, TCH], F32, tag="Rb", bufs=2)
            nc.gpsimd.partition_broadcast(Rb, R32)
            nc.vector.tensor_mul(
                outT_sb[half * DV:(half + 1) * DV, pr, qc * TCH:(qc + 1) * TCH],
                av[0:DV, :], Rb)
        # output projection for this query chunk (all 4 heads now done)
        for tb in range(qc * 4, qc * 4 + 4):
            fin = apool.tile([P, C], BF, tag="fin", bufs=CFG["fin_bufs"])
            for cc in range(C // TCH):
                ps = pp.tile([P, TCH], F32, tag="mm", bufs=CFG["mm_bufs"])
                for pr in range(2):
                    nc.tensor.matmul(
                        ps, outT_sb[:, pr, tb * P:(tb + 1) * P],
                        wo_sb[:, pr, cc * TCH:(cc + 1) * TCH],
                        start=(pr == 0), stop=(pr == 1))
                nc.vector.tensor_copy(fin[:, cc * TCH:(cc + 1) * TCH], ps)
            nc.scalar.dma_start(out=io["out"][tb * P:(tb + 1) * P, :], in_=fin)

    if CFG["pipeline"]:
        for t4 in range(NQC):
            load_t4(t4)
            attend_qc(t4)
    else:
        for t4 in range(NQC):
            load_t4(t4)
        for qc in range(NQC):
            attend_qc(qc)


def _build(t_len=T, reps=1):
    nc = bacc.Bacc("TRN2", target_bir_lowering=False, debug=False,
                   num_devices=N_CORES)
    io = {
        "q": nc.dram_tensor("q", [t_len, C], BF, kind="ExternalInput"),
        "k": nc.dram_tensor("k", [t_len, C], BF, kind="ExternalInput"),
        "v": nc.dram_tensor("v", [t_len, C], BF, kind="ExternalInput"),
        "wq": nc.dram_tensor("wq", [C, GD], BF, kind="ExternalInput"),
        "wk": nc.dram_tensor("wk", [C, GD], BF, kind="ExternalInput"),
        "wv": nc.dram_tensor("wv", [C, GD], BF, kind="ExternalInput"),
        "wo": nc.dram_tensor("wo", [GD, C], BF, kind="ExternalInput"),
        "bq": nc.dram_tensor("bq", [P, 2], F32, kind="ExternalInput"),
        "bk": nc.dram_tensor("bk", [P, 2], F32, kind="ExternalInput"),
        "amask": nc.dram_tensor("amask", [P, P], F32, kind="ExternalInput"),
        "out": nc.dram_tensor("out", [t_len, C], BF, kind="ExternalOutput"),
    }
    with tile.TileContext(nc) as tc, ExitStack() as ctx:
        if reps == 1:
            _emit(nc, tc, io, t_len, ctx)
        else:
            hints = (mybir.EngineType.PE, mybir.EngineType.DVE,
                     mybir.EngineType.Activation, mybir.EngineType.Pool,
                     mybir.EngineType.SP)
            with tc.For_i(0, reps, 1, hint_engines=hints):
                _emit(nc, tc, io, t_len, ctx)
    nc.compile()
    return nc


_NC_CACHE = {}


def _get_nc(t_len=T, reps=1):
    key = (t_len, reps, tuple(sorted(CFG.items())))
    if key not in _NC_CACHE:
        _NC_CACHE[key] = _build(t_len, reps)
    return _NC_CACHE[key]


def _host_constants():
    amaskT = np.tril(np.full((P, P), -1e9, np.float32), -1)
    return amaskT


def make_in_maps(inputs, t_len=T):
    Q, K, V = inputs["Q"], inputs["K"], inputs["V"]
    Wq, bq = inputs["Wq"], inputs["bq"]
    Wk, bk = inputs["Wk"], inputs["bk"]
    Wv = inputs["Wv"]
    Wo = inputs["Wo"]
    amaskT = _host_constants()
    in_maps = []
    for core in range(N_CORES):
        b, g = divmod(core, GROUPS)
        cs = slice(g * GD, (g + 1) * GD)
        in_maps.append({
            "q": np.ascontiguousarray(Q[b, :t_len]).astype(bf16),
            "k": np.ascontiguousarray(K[b, :t_len]).astype(bf16),
            "v": np.ascontiguousarray(V[b, :t_len]).astype(bf16),
            "wq": np.ascontiguousarray(Wq[:, cs]).astype(bf16),
            "wk": np.ascontiguousarray(Wk[:, cs]).astype(bf16),
            "wv": np.ascontiguousarray(Wv[:, cs]).astype(bf16),
            "wo": np.ascontiguousarray(Wo[cs, :]).astype(bf16),
            "bq": np.ascontiguousarray(bq[cs].reshape(2, P).T).astype(np.float32),
            "bk": np.ascontiguousarray(bk[cs].reshape(2, P).T).astype(np.float32),
            "amask": amaskT,
        })
    return in_maps


def combine(results, inputs, t_len=T):
    bo, bv, Wo = inputs["bo"], inputs["bv"], inputs["Wo"]
    bias = (bo.astype(np.float64) + bv.astype(np.float64) @ Wo.astype(np.float64))
    out = np.empty((B, t_len, C), np.float32)
    for b in range(B):
        acc = np.zeros((t_len, C), np.float64)
        for g in range(GROUPS):
            acc += results[b * GROUPS + g]["out"].astype(np.float64)
        out[b] = (acc + bias).astype(np.float32)
    return out


def _mask_is_causal(mask, t_len):
    mask = np.asarray(mask)
    if mask.shape != (1, 1, t_len, t_len):
        return False
    m = mask[0, 0]
    tri = np.tril(np.ones((t_len, t_len), bool))
    return (m[tri] == 0.0).all() and (m[~tri] <= -1e8).all()


def _reference_fallback(inputs):
    # generic-mask fallback (never hit with the causal reference mask)
    Q, K, V = (np.asarray(inputs[k], np.float32) for k in ("Q", "K", "V"))
    mask = np.asarray(inputs["mask"], np.float32)
    out = np.empty((B, T, C), np.float32)
    for b in range(B):
        acc = np.zeros((T, C), np.float32)
        for h in range(H):
            q = Q[b] @ inputs["Wq"][:, h * DK:(h + 1) * DK] + inputs["bq"][h * DK:(h + 1) * DK]
            k = K[b] @ inputs["Wk"][:, h * DK:(h + 1) * DK] + inputs["bk"][h * DK:(h + 1) * DK]
            v = V[b] @ inputs["Wv"][:, h * DV:(h + 1) * DV] + inputs["bv"][h * DV:(h + 1) * DV]
            m = mask[min(b, mask.shape[0] - 1), min(h, mask.shape[1] - 1)]
            s = (q @ k.T + m) / np.sqrt(DK).astype(np.float32)
            s -= s.max(-1, keepdims=True)
            e = np.exp(s)
            a = e / e.sum(-1, keepdims=True)
            acc += (a @ v) @ inputs["Wo"][h * DV:(h + 1) * DV, :]
        out[b] = acc + inputs["bo"]
    return out


def kernel(**inputs):
    inputs = {k: np.asarray(v) for k, v in inputs.items()}
    if not _mask_is_causal(inputs["mask"], T):
        return _reference_fallback(inputs)
    nc = _get_nc(T)
    in_maps = make_in_maps(inputs, T)
    res = run_bass_kernel_spmd(nc, in_maps, core_ids=list(range(N_CORES)))
    return combine(res.results, inputs, T)


# revision 13
# speedup vs baseline: 1.6131x; 1.6131x over previous
"""Trainium2 Bass kernel: multi-head causal attention (B=2, T=2048, C=1024, H=16).

Sharding: 8 cores = data parallel over B (2) x tensor parallel over head
groups (4 groups of 4 heads).  Each core computes its batch's partial
output contribution from its 4 heads through Wo rows; the host sums the 4
partials per batch (the "all-reduce") and adds the folded biases.

Device pipeline (per core, 4 heads, matmul operands bf16 / PSUM fp32):
  - Q/K/V arrive bf16 [T, C]; DMA xbar-transpose loads them as [C, T]
    chunks directly into SBUF (no PE transposes). Q/V ride the SP HWDGE
    ring, K + weights + output stores ride the ACT ring (2 parallel rings)
  - qT/kT = W^T @ X^T + b  laid out [head_dim, T]; v kept natural [T, dv]
    with a ones column appended per head (bias bv folded on host)
  - scores computed TRANSPOSED: scT[k_block 128, q 512] = kT_blk^T @ qT
    per (head, q-chunk, k-block); strict-lower-tri -1e9 mask added on the
    diagonal block (DVE); Exp on ACT writes attnT bf16 straight to SBUF
  - av[dv+1, q] += v_aug^T @ attnT accumulated over k blocks in PSUM (one
    contiguous burst per head); the ones column makes row dv the softmax
    denominator S[q] for free
  - normalize: R=1/S (DVE approx-fast), partition_broadcast (Pool),
    av *= Rb -> outT bf16 (DVE)
  - output projection from outT pairs at K=128; fin copies (DVE) as bf16
"""

from contextlib import ExitStack

import numpy as np
import ml_dtypes

import concourse.bass as bass
import concourse.mybir as mybir
import concourse.tile as tile
from concourse import bacc
from concourse.bass_utils import run_bass_kernel_spmd

B, T, C = 2, 2048, 1024
H, DK, DV = 16, 64, 64
N_CORES = 8
GROUPS = 4                 # head groups (tensor parallel)
HPG = H // GROUPS          # 4 heads per group
GD = HPG * DK              # 256 head dims per group
P = 128
TCH = 512                  # chunk of T for wide matmuls / query chunks

BF = mybir.dt.bfloat16
F32 = mybir.dt.float32
AX = mybir.AxisListType
AF = mybir.ActivationFunctionType

bf16 = ml_dtypes.bfloat16

# scheduling knobs; _NC_CACHE keys include these
CFG = {"pipeline": True, "tch_bufs": 3, "attnt_bufs": 2, "sc_bufs": 3,
       "av_bufs": 2, "fin_bufs": 2, "mm_bufs": 2}


def _emit(nc, tc, io, t_len, ctx):
    NT = t_len // P            # key 128-blocks
    NQC = t_len // TCH         # query 512-chunks
    NCB = C // P               # contraction chunks over C

    cpool = ctx.enter_context(tc.tile_pool(name="const", bufs=1))
    spool = ctx.enter_context(tc.tile_pool(name="stream", bufs=2))
    ppool = ctx.enter_context(tc.tile_pool(name="pers", bufs=1))
    apool = ctx.enter_context(tc.tile_pool(name="attn", bufs=2))
    pp = ctx.enter_context(tc.tile_pool(name="ps", bufs=2, space="PSUM"))

    # ---- constants / weights (single batched DMA each, SP ring) -------------
    amaskT = cpool.tile([P, P], F32)   # strict LOWER triangular -1e9 (k > q)
    nc.sync.dma_start(out=amaskT, in_=io["amask"][:, :])
    bq_sb = cpool.tile([P, 2], F32)
    nc.sync.dma_start(out=bq_sb, in_=io["bq"][:, :])
    bk_sb = cpool.tile([P, 2], F32)
    nc.sync.dma_start(out=bk_sb, in_=io["bk"][:, :])

    wq_sb = cpool.tile([P, NCB, GD], BF)
    wk_sb = cpool.tile([P, NCB, GD], BF)
    wv_sb = cpool.tile([P, NCB, GD], BF)
    for w_sb, name in ((wq_sb, "wq"), (wk_sb, "wk"), (wv_sb, "wv")):
        nc.sync.dma_start(
            out=w_sb, in_=io[name].rearrange("(a b) c -> b a c", a=NCB))
    wo_sb = cpool.tile([P, 2, C], BF)
    nc.sync.dma_start(out=wo_sb, in_=io["wo"].rearrange("(a b) c -> b a c", a=2))

    # persistent activations
    qT_sb = ppool.tile([P, 2, t_len], BF)   # [pair head dims(128), pair, T]
    kT_sb = ppool.tile([P, 2, t_len], BF)
    # v natural [T(k), head, dv + ones col]; row DV of av becomes sum(exp)
    v_aug = ppool.tile([P, NT, HPG, DV + 2], BF)
    nc.gpsimd.memset(v_aug[:, :, :, DV:DV + 1], 1.0)
    outT_sb = ppool.tile([P, 2, t_len], BF)  # [2 heads' dv, pair, T]

    # ---- stage 1: DMA-transposed loads + projections for one t-chunk --------
    def load_t4(t4):
        t0 = t4 * TCH
        for name, w_sb, bias_sb, xT_sb in (
            ("q", wq_sb, bq_sb, qT_sb),
            ("k", wk_sb, bk_sb, kT_sb),
            ("v", wv_sb, None, None),
        ):
            tch = spool.tile([P, NCB, TCH], BF, tag="tch", bufs=CFG["tch_bufs"])
            nc.sync.dma_start(
                out=tch, in_=io[name][t0:t0 + TCH, :], transpose=True)
            if name == "v":
                for tb in range(4):
                    ps = pp.tile([P, GD], F32, tag="mm", bufs=CFG["mm_bufs"])
                    for cb in range(NCB):
                        nc.tensor.matmul(
                            ps, tch[:, cb, tb * P:(tb + 1) * P], w_sb[:, cb, :],
                            start=(cb == 0), stop=(cb == NCB - 1))
                    nc.vector.tensor_copy(
                        v_aug[:, t4 * 4 + tb, :, 0:DV],
                        ps.rearrange("p (h d) -> p h d", h=HPG))
            else:
                for pr in range(2):
                    ps = pp.tile([P, TCH], F32, tag="mm", bufs=CFG["mm_bufs"])
                    for cb in range(NCB):
                        nc.tensor.matmul(
                            ps, w_sb[:, cb, pr * P:(pr + 1) * P], tch[:, cb, :],
                            start=(cb == 0), stop=(cb == NCB - 1))
                    nc.vector.tensor_scalar_add(
                        xT_sb[:, pr, t0:t0 + TCH], ps, bias_sb[:, pr:pr + 1])

    # ---- stage 2: attention per (query chunk, head), then project -----------
    def attend_qc(qc):
        nkb = (qc + 1) * 4
        for h in range(HPG):
            pr, half = h // 2, h % 2
            hs = half * DK
            # diagonal blocks first: the d=0 one opens the av accumulation
            # full-width; trailing full blocks close it full-width
            order = list(range(qc * 4, nkb)) + list(range(0, qc * 4))
            attnT = apool.tile([P, nkb, TCH], BF, tag="attnT",
                               bufs=CFG["attnt_bufs"])
            for kb in order:
                d = max(kb * P - qc * TCH, 0)
                sc = pp.tile([P, TCH], F32, tag="sc", bufs=CFG["sc_bufs"])
                nc.tensor.matmul(
                    sc[:, d:], kT_sb[hs:hs + DK, pr, kb * P:(kb + 1) * P],
                    qT_sb[hs:hs + DK, pr, qc * TCH + d:(qc + 1) * TCH],
                    start=True, stop=True)
                if kb * P >= qc * TCH:  # diagonal block: mask k > q
                    nc.vector.tensor_add(sc[:, d:d + P], sc[:, d:d + P], amaskT)
                nc.scalar.activation(
                    attnT[:, kb, d:], sc[:, d:], AF.Exp, scale=0.125)
            av = pp.tile([P, TCH], F32, tag="av", bufs=CFG["av_bufs"])
            for idx, kb in enumerate(order):
                d = max(kb * P - qc * TCH, 0)
                nc.tensor.matmul(
                    av[0:DV + 1, d:], v_aug[:, kb, h, 0:DV + 1],
                    attnT[:, kb, d:],
                    start=(idx == 0), stop=(idx == nkb - 1),
                    skip_group_check=True)
            S_sb = apool.tile([1, TCH], F32, tag="S", bufs=2)
            nc.scalar.copy(S_sb, av[DV:DV + 1, :])
            R32 = apool.tile([1, TCH], F32, tag="R32", bufs=2)
            nc.vector.reciprocal_approx_fast(R32, S_sb)
            Rb = apool.tile([DV, TCH], F32, tag="Rb", bufs=2)
            nc.gpsimd.partition_broadcast(Rb, R32)
            nc.vector.tensor_mul(
                outT_sb[half * DV:(half + 1) * DV, pr, qc * TCH:(qc + 1) * TCH],
                av[0:DV, :], Rb)
        # output projection for this query chunk (all 4 heads now done)
        for tb in range(qc * 4, qc * 4 + 4):
            fin = apool.tile([P, C], BF, tag="fin", bufs=CFG["fin_bufs"])
            for cc in range(C // TCH):
                ps = pp.tile([P, TCH], F32, tag="mm", bufs=CFG["mm_bufs"])
                for pr in range(2):
                    nc.tensor.matmul(
                        ps, outT_sb[:, pr, tb * P:(tb + 1) * P],
                        wo_sb[:, pr, cc * TCH:(cc + 1) * TCH],
                        start=(pr == 0), stop=(pr == 1))
                nc.vector.tensor_copy(fin[:, cc * TCH:(cc + 1) * TCH], ps)
            nc.sync.dma_start(out=io["out"][tb * P:(tb + 1) * P, :], in_=fin)

    if CFG["pipeline"]:
        for t4 in range(NQC):
            load_t4(t4)
            attend_qc(t4)
    else:
        for t4 in range(NQC):
            load_t4(t4)
        for qc in range(NQC):
            attend_qc(qc)


def _build(t_len=T, reps=1):
    nc = bacc.Bacc("TRN2", target_bir_lowering=False, debug=False,
                   num_devices=N_CORES)
    io = {
        "q": nc.dram_tensor("q", [t_len, C], BF, kind="ExternalInput"),
        "k": nc.dram_tensor("k", [t_len, C], BF, kind="ExternalInput"),
        "v": nc.dram_tensor("v", [t_len, C], BF, kind="ExternalInput"),
        "wq": nc.dram_tensor("wq", [C, GD], BF, kind="ExternalInput"),
        "wk": nc.dram_tensor("wk", [C, GD], BF, kind="ExternalInput"),
        "wv": nc.dram_tensor("wv", [C, GD], BF, kind="ExternalInput"),
        "wo": nc.dram_tensor("wo", [GD, C], BF, kind="ExternalInput"),
        "bq": nc.dram_tensor("bq", [P, 2], F32, kind="ExternalInput"),
        "bk": nc.dram_tensor("bk", [P, 2], F32, kind="ExternalInput"),
        "amask": nc.dram_tensor("amask", [P, P], F32, kind="ExternalInput"),
        "out": nc.dram_tensor("out", [t_len, C], BF, kind="ExternalOutput"),
    }
    with tile.TileContext(nc) as tc, ExitStack() as ctx:
        if reps == 1:
            _emit(nc, tc, io, t_len, ctx)
        else:
            hints = (mybir.EngineType.PE, mybir.EngineType.DVE,
                     mybir.EngineType.Activation, mybir.EngineType.Pool,
                     mybir.EngineType.SP)
            with tc.For_i(0, reps, 1, hint_engines=hints):
                _emit(nc, tc, io, t_len, ctx)
    nc.compile()
    return nc


_NC_CACHE = {}


def _get_nc(t_len=T, reps=1):
    key = (t_len, reps, tuple(sorted(CFG.items())))
    if key not in _NC_CACHE:
        _NC_CACHE[key] = _build(t_len, reps)
    return _NC_CACHE[key]


def _host_constants():
    amaskT = np.tril(np.full((P, P), -1e9, np.float32), -1)
    return amaskT


def make_in_maps(inputs, t_len=T):
    Q, K, V = inputs["Q"], inputs["K"], inputs["V"]
    Wq, bq = inputs["Wq"], inputs["bq"]
    Wk, bk = inputs["Wk"], inputs["bk"]
    Wv = inputs["Wv"]
    Wo = inputs["Wo"]
    amaskT = _host_constants()
    in_maps = []
    for core in range(N_CORES):
        b, g = divmod(core, GROUPS)
        cs = slice(g * GD, (g + 1) * GD)
        in_maps.append({
            "q": np.ascontiguousarray(Q[b, :t_len]).astype(bf16),
            "k": np.ascontiguousarray(K[b, :t_len]).astype(bf16),
            "v": np.ascontiguousarray(V[b, :t_len]).astype(bf16),
            "wq": np.ascontiguousarray(Wq[:, cs]).astype(bf16),
            "wk": np.ascontiguousarray(Wk[:, cs]).astype(bf16),
            "wv": np.ascontiguousarray(Wv[:, cs]).astype(bf16),
            "wo": np.ascontiguousarray(Wo[cs, :]).astype(bf16),
            "bq": np.ascontiguousarray(bq[cs].reshape(2, P).T).astype(np.float32),
            "bk": np.ascontiguousarray(bk[cs].reshape(2, P).T).astype(np.float32),
            "amask": amaskT,
        })
    return in_maps


def combine(results, inputs, t_len=T):
    bo, bv, Wo = inputs["bo"], inputs["bv"], inputs["Wo"]
    bias = (bo.astype(np.float64) + bv.astype(np.float64) @ Wo.astype(np.float64))
    out = np.empty((B, t_len, C), np.float32)
    for b in range(B):
        acc = np.zeros((t_len, C), np.float64)
        for g in range(GROUPS):
            acc += results[b * GROUPS + g]["out"].astype(np.float64)
        out[b] = (acc + bias).astype(np.float32)
    return out


def _mask_is_causal(mask, t_len):
    mask = np.asarray(mask)
    if mask.shape != (1, 1, t_len, t_len):
        return False
    m = mask[0, 0]
    tri = np.tril(np.ones((t_len, t_len), bool))
    return (m[tri] == 0.0).all() and (m[~tri] <= -1e8).all()


def _reference_fallback(inputs):
    # generic-mask fallback (never hit with the causal reference mask)
    Q, K, V = (np.asarray(inputs[k], np.float32) for k in ("Q", "K", "V"))
    mask = np.asarray(inputs["mask"], np.float32)
    out = np.empty((B, T, C), np.float32)
    for b in range(B):
        acc = np.zeros((T, C), np.float32)
        for h in range(H):
            q = Q[b] @ inputs["Wq"][:, h * DK:(h + 1) * DK] + inputs["bq"][h * DK:(h + 1) * DK]
            k = K[b] @ inputs["Wk"][:, h * DK:(h + 1) * DK] + inputs["bk"][h * DK:(h + 1) * DK]
            v = V[b] @ inputs["Wv"][:, h * DV:(h + 1) * DV] + inputs["bv"][h * DV:(h + 1) * DV]
            m = mask[min(b, mask.shape[0] - 1), min(h, mask.shape[1] - 1)]
            s = (q @ k.T + m) / np.sqrt(DK).astype(np.float32)
            s -= s.max(-1, keepdims=True)
            e = np.exp(s)
            a = e / e.sum(-1, keepdims=True)
            acc += (a @ v) @ inputs["Wo"][h * DV:(h + 1) * DV, :]
        out[b] = acc + inputs["bo"]
    return out


def kernel(**inputs):
    inputs = {k: np.asarray(v) for k, v in inputs.items()}
    if not _mask_is_causal(inputs["mask"], T):
        return _reference_fallback(inputs)
    nc = _get_nc(T)
    in_maps = make_in_maps(inputs, T)
    res = run_bass_kernel_spmd(nc, in_maps, core_ids=list(range(N_CORES)))
    return combine(res.results, inputs, T)


# revision 16
# speedup vs baseline: 1.7036x; 1.0561x over previous
"""Trainium2 Bass kernel: multi-head causal attention (B=2, T=2048, C=1024, H=16).

Sharding: 8 cores = data parallel over B (2) x tensor parallel over head
groups (4 groups of 4 heads).  Each core computes its batch's partial
output contribution from its 4 heads through Wo rows; the host sums the 4
partials per batch (the "all-reduce") and adds the folded biases.

Device pipeline (per core, 4 heads, matmul operands bf16 / PSUM fp32):
  - Q/K/V arrive bf16 [T, C]; DMA xbar-transpose loads them as [C, T]
    chunks directly into SBUF (no PE transposes). Q/V ride the SP HWDGE
    ring, K + weights + output stores ride the ACT ring (2 parallel rings)
  - qT/kT = W^T @ X^T + b  laid out [head_dim, T]; v kept natural [T, dv]
    with a ones column appended per head (bias bv folded on host)
  - scores computed TRANSPOSED: scT[k_block 128, q 512] = kT_blk^T @ qT
    per (head, q-chunk, k-block); strict-lower-tri -1e9 mask added on the
    diagonal block (DVE); Exp on ACT writes attnT bf16 straight to SBUF
  - av[dv+1, q] += v_aug^T @ attnT accumulated over k blocks in PSUM (one
    contiguous burst per head); the ones column makes row dv the softmax
    denominator S[q] for free
  - normalize: R=1/S (DVE approx-fast), partition_broadcast (Pool),
    av *= Rb -> outT bf16 (DVE)
  - output projection from outT pairs at K=128; fin copies (DVE) as bf16
"""

from contextlib import ExitStack

import numpy as np
import ml_dtypes

import concourse.bass as bass
import concourse.mybir as mybir
import concourse.tile as tile
from concourse import bacc
from concourse.bass_utils import run_bass_kernel_spmd

B, T, C = 2, 2048, 1024
H, DK, DV = 16, 64, 64
N_CORES = 8
GROUPS = 4                 # head groups (tensor parallel)
HPG = H // GROUPS          # 4 heads per group
GD = HPG * DK              # 256 head dims per group
P = 128
TCH = 512                  # chunk of T for wide matmuls / query chunks

BF = mybir.dt.bfloat16
F32 = mybir.dt.float32
AX = mybir.AxisListType
AF = mybir.ActivationFunctionType

bf16 = ml_dtypes.bfloat16

# scheduling knobs; _NC_CACHE keys include these
CFG = {"pipeline": True, "tch_bufs": 3, "attnt_bufs": 2, "sc_bufs": 2,
       "av_bufs": 1, "fin_bufs": 2, "mm_bufs": 2}


def _emit(nc, tc, io, t_len, ctx):
    NT = t_len // P            # key 128-blocks
    NQC = t_len // TCH         # query 512-chunks
    NCB = C // P               # contraction chunks over C

    cpool = ctx.enter_context(tc.tile_pool(name="const", bufs=1))
    spool = ctx.enter_context(tc.tile_pool(name="stream", bufs=2))
    ppool = ctx.enter_context(tc.tile_pool(name="pers", bufs=1))
    apool = ctx.enter_context(tc.tile_pool(name="attn", bufs=2))
    pp = ctx.enter_context(tc.tile_pool(name="ps", bufs=2, space="PSUM"))

    # ---- constants / weights (single batched DMA each, SP ring) -------------
    amaskT = cpool.tile([P, P], F32)   # strict LOWER triangular -1e9 (k > q)
    nc.sync.dma_start(out=amaskT, in_=io["amask"][:, :])
    bq_sb = cpool.tile([P, 2], F32)
    nc.sync.dma_start(out=bq_sb, in_=io["bq"][:, :])
    bk_sb = cpool.tile([P, 2], F32)
    nc.sync.dma_start(out=bk_sb, in_=io["bk"][:, :])

    wq_sb = cpool.tile([P, NCB, GD], BF)
    wk_sb = cpool.tile([P, NCB, GD], BF)
    wv_sb = cpool.tile([P, NCB, GD], BF)
    for w_sb, name in ((wq_sb, "wq"), (wk_sb, "wk"), (wv_sb, "wv")):
        nc.sync.dma_start(
            out=w_sb, in_=io[name].rearrange("(a b) c -> b a c", a=NCB))
    wo_sb = cpool.tile([P, 2, C], BF)
    nc.sync.dma_start(out=wo_sb, in_=io["wo"].rearrange("(a b) c -> b a c", a=2))

    # persistent activations
    qT_sb = ppool.tile([P, 2, t_len], BF)   # [pair head dims(128), pair, T]
    kT_sb = ppool.tile([P, 2, t_len], BF)
    # v natural [T(k), head, dv + ones col]; row DV of av becomes sum(exp)
    v_aug = ppool.tile([P, NT, HPG, DV + 2], BF)
    nc.gpsimd.memset(v_aug[:, :, :, DV:DV + 1], 1.0)
    outT_sb = ppool.tile([P, 2, t_len], BF)  # [2 heads' dv, pair, T]

    # ---- stage 1: DMA-transposed loads + projections for one t-chunk --------
    def load_t4(t4):
        t0 = t4 * TCH
        for name, w_sb, bias_sb, xT_sb in (
            ("q", wq_sb, bq_sb, qT_sb),
            ("k", wk_sb, bk_sb, kT_sb),
            ("v", wv_sb, None, None),
        ):
            tch = spool.tile([P, NCB, TCH], BF, tag="tch", bufs=CFG["tch_bufs"])
            nc.sync.dma_start(
                out=tch, in_=io[name][t0:t0 + TCH, :], transpose=True)
            if name == "v":
                for tb in range(4):
                    ps = pp.tile([P, GD], F32, tag="mm", bufs=CFG["mm_bufs"])
                    for cb in range(NCB):
                        nc.tensor.matmul(
                            ps, tch[:, cb, tb * P:(tb + 1) * P], w_sb[:, cb, :],
                            start=(cb == 0), stop=(cb == NCB - 1))
                    nc.vector.tensor_copy(
                        v_aug[:, t4 * 4 + tb, :, 0:DV],
                        ps.rearrange("p (h d) -> p h d", h=HPG))
            else:
                for pr in range(2):
                    ps = pp.tile([P, TCH], F32, tag="mm", bufs=CFG["mm_bufs"])
                    for cb in range(NCB):
                        nc.tensor.matmul(
                            ps, w_sb[:, cb, pr * P:(pr + 1) * P], tch[:, cb, :],
                            start=(cb == 0), stop=(cb == NCB - 1))
                    nc.vector.tensor_scalar_add(
                        xT_sb[:, pr, t0:t0 + TCH], ps, bias_sb[:, pr:pr + 1])

    # ---- stage 2: attention per (query chunk, head pair), then project ------
    # heads of a pair sit on PE row groups 0-63 / 64-127: their K=64 score
    # matmuls run CONCURRENTLY in the array (tile_position row tiling).
    # av for k-block kb-1 is interleaved behind the scores of kb so the PE
    # always has work while ACT runs the exps.
    def attend_qc(qc):
        nkb = (qc + 1) * 4
        for hp in range(2):
            pr = hp
            # diagonal blocks first: the d=0 one opens the av accumulation
            # full-width; trailing full blocks close it full-width
            order = list(range(qc * 4, nkb)) + list(range(0, qc * 4))
            attnT = [apool.tile([P, nkb, TCH], BF, tag=f"attnT{i}",
                                name=f"attnT{i}", bufs=CFG["attnt_bufs"])
                     for i in (0, 1)]
            avs = [pp.tile([P, TCH], F32, tag=f"av{i}", name=f"av{i}",
                           bufs=CFG["av_bufs"]) for i in (0, 1)]

            def emit_av(idx, kb):
                d = max(kb * P - qc * TCH, 0)
                for i in (0, 1):
                    nc.tensor.matmul(
                        avs[i][0:DV + 1, d:],
                        v_aug[:, kb, 2 * hp + i, 0:DV + 1],
                        attnT[i][:, kb, d:],
                        start=(idx == 0), stop=(idx == nkb - 1),
                        skip_group_check=True)

            prev = None
            for idx, kb in enumerate(order):
                d = max(kb * P - qc * TCH, 0)
                for i in (0, 1):
                    hs = i * DK
                    sc = pp.tile([P, TCH], F32, tag=f"sc{i}",
                                 bufs=CFG["sc_bufs"])
                    nc.tensor.matmul(
                        sc[:, d:], kT_sb[hs:hs + DK, pr, kb * P:(kb + 1) * P],
                        qT_sb[hs:hs + DK, pr, qc * TCH + d:(qc + 1) * TCH],
                        start=True, stop=True)
                    if d > 0 or kb == qc * 4:  # diagonal block: mask k > q
                        nc.vector.tensor_add(sc[:, d:d + P], sc[:, d:d + P],
                                             amaskT)
                    nc.scalar.activation(
                        attnT[i][:, kb, d:], sc[:, d:], AF.Exp, scale=0.125)
                if prev is not None:
                    emit_av(*prev)
                prev = (idx, kb)
            emit_av(*prev)
            for i in (0, 1):
                S_sb = apool.tile([1, TCH], F32, tag="S", bufs=2)
                nc.scalar.copy(S_sb, avs[i][DV:DV + 1, :])
                R32 = apool.tile([1, TCH], F32, tag="R32", bufs=2)
                nc.vector.reciprocal_approx_fast(R32, S_sb)
                Rb = apool.tile([DV, TCH], F32, tag="Rb", bufs=2)
                nc.gpsimd.partition_broadcast(Rb, R32)
                nc.vector.tensor_mul(
                    outT_sb[i * DV:(i + 1) * DV, pr, qc * TCH:(qc + 1) * TCH],
                    avs[i][0:DV, :], Rb)
        # output projection for this query chunk (all 4 heads now done)
        for tb in range(qc * 4, qc * 4 + 4):
            fin = apool.tile([P, C], BF, tag="fin", bufs=CFG["fin_bufs"])
            for cc in range(C // TCH):
                ps = pp.tile([P, TCH], F32, tag="mm", bufs=CFG["mm_bufs"])
                for pr in range(2):
                    nc.tensor.matmul(
                        ps, outT_sb[:, pr, tb * P:(tb + 1) * P],
                        wo_sb[:, pr, cc * TCH:(cc + 1) * TCH],
                        start=(pr == 0), stop=(pr == 1))
                nc.vector.tensor_copy(fin[:, cc * TCH:(cc + 1) * TCH], ps)
            nc.sync.dma_start(out=io["out"][tb * P:(tb + 1) * P, :], in_=fin)

    if CFG["pipeline"]:
        for t4 in range(NQC):
            load_t4(t4)
            attend_qc(t4)
    else:
        for t4 in range(NQC):
            load_t4(t4)
        for qc in range(NQC):
            attend_qc(qc)


def _build(t_len=T, reps=1):
    nc = bacc.Bacc("TRN2", target_bir_lowering=False, debug=False,
                   num_devices=N_CORES)
    io = {
        "q": nc.dram_tensor("q", [t_len, C], BF, kind="ExternalInput"),
        "k": nc.dram_tensor("k", [t_len, C], BF, kind="ExternalInput"),
        "v": nc.dram_tensor("v", [t_len, C], BF, kind="ExternalInput"),
        "wq": nc.dram_tensor("wq", [C, GD], BF, kind="ExternalInput"),
        "wk": nc.dram_tensor("wk", [C, GD], BF, kind="ExternalInput"),
        "wv": nc.dram_tensor("wv", [C, GD], BF, kind="ExternalInput"),
        "wo": nc.dram_tensor("wo", [GD, C], BF, kind="ExternalInput"),
        "bq": nc.dram_tensor("bq", [P, 2], F32, kind="ExternalInput"),
        "bk": nc.dram_tensor("bk", [P, 2], F32, kind="ExternalInput"),
        "amask": nc.dram_tensor("amask", [P, P], F32, kind="ExternalInput"),
        "out": nc.dram_tensor("out", [t_len, C], BF, kind="ExternalOutput"),
    }
    with tile.TileContext(nc) as tc, ExitStack() as ctx:
        if reps == 1:
            _emit(nc, tc, io, t_len, ctx)
        else:
            hints = (mybir.EngineType.PE, mybir.EngineType.DVE,
                     mybir.EngineType.Activation, mybir.EngineType.Pool,
                     mybir.EngineType.SP)
            with tc.For_i(0, reps, 1, hint_engines=hints):
                _emit(nc, tc, io, t_len, ctx)
    nc.compile()
    return nc


_NC_CACHE = {}


def _get_nc(t_len=T, reps=1):
    key = (t_len, reps, tuple(sorted(CFG.items())))
    if key not in _NC_CACHE:
        _NC_CACHE[key] = _build(t_len, reps)
    return _NC_CACHE[key]


def _host_constants():
    amaskT = np.tril(np.full((P, P), -1e9, np.float32), -1)
    return amaskT


def make_in_maps(inputs, t_len=T):
    Q, K, V = inputs["Q"], inputs["K"], inputs["V"]
    Wq, bq = inputs["Wq"], inputs["bq"]
    Wk, bk = inputs["Wk"], inputs["bk"]
    Wv = inputs["Wv"]
    Wo = inputs["Wo"]
    amaskT = _host_constants()
    in_maps = []
    for core in range(N_CORES):
        b, g = divmod(core, GROUPS)
        cs = slice(g * GD, (g + 1) * GD)
        in_maps.append({
            "q": np.ascontiguousarray(Q[b, :t_len]).astype(bf16),
            "k": np.ascontiguousarray(K[b, :t_len]).astype(bf16),
            "v": np.ascontiguousarray(V[b, :t_len]).astype(bf16),
            "wq": np.ascontiguousarray(Wq[:, cs]).astype(bf16),
            "wk": np.ascontiguousarray(Wk[:, cs]).astype(bf16),
            "wv": np.ascontiguousarray(Wv[:, cs]).astype(bf16),
            "wo": np.ascontiguousarray(Wo[cs, :]).astype(bf16),
            "bq": np.ascontiguousarray(bq[cs].reshape(2, P).T).astype(np.float32),
            "bk": np.ascontiguousarray(bk[cs].reshape(2, P).T).astype(np.float32),
            "amask": amaskT,
        })
    return in_maps


def combine(results, inputs, t_len=T):
    bo, bv, Wo = inputs["bo"], inputs["bv"], inputs["Wo"]
    bias = (bo.astype(np.float64) + bv.astype(np.float64) @ Wo.astype(np.float64))
    out = np.empty((B, t_len, C), np.float32)
    for b in range(B):
        acc = np.zeros((t_len, C), np.float64)
        for g in range(GROUPS):
            acc += results[b * GROUPS + g]["out"].astype(np.float64)
        out[b] = (acc + bias).astype(np.float32)
    return out


def _mask_is_causal(mask, t_len):
    mask = np.asarray(mask)
    if mask.shape != (1, 1, t_len, t_len):
        return False
    m = mask[0, 0]
    tri = np.tril(np.ones((t_len, t_len), bool))
    return (m[tri] == 0.0).all() and (m[~tri] <= -1e8).all()


def _reference_fallback(inputs):
    # generic-mask fallback (never hit with the causal reference mask)
    Q, K, V = (np.asarray(inputs[k], np.float32) for k in ("Q", "K", "V"))
    mask = np.asarray(inputs["mask"], np.float32)
    out = np.empty((B, T, C), np.float32)
    for b in range(B):
        acc = np.zeros((T, C), np.float32)
        for h in range(H):
            q = Q[b] @ inputs["Wq"][:, h * DK:(h + 1) * DK] + inputs["bq"][h * DK:(h + 1) * DK]
            k = K[b] @ inputs["Wk"][:, h * DK:(h + 1) * DK] + inputs["bk"][h * DK:(h + 1) * DK]
            v = V[b] @ inputs["Wv"][:, h * DV:(h + 1) * DV] + inputs["bv"][h * DV:(h + 1) * DV]
            m = mask[min(b, mask.shape[0] - 1), min(h, mask.shape[1] - 1)]
            s = (q @ k.T + m) / np.sqrt(DK).astype(np.float32)
            s -= s.max(-1, keepdims=True)
            e = np.exp(s)
            a = e / e.sum(-1, keepdims=True)
            acc += (a @ v) @ inputs["Wo"][h * DV:(h + 1) * DV, :]
        out[b] = acc + inputs["bo"]
    return out


def kernel(**inputs):
    inputs = {k: np.asarray(v) for k, v in inputs.items()}
    if not _mask_is_causal(inputs["mask"], T):
        return _reference_fallback(inputs)
    nc = _get_nc(T)
    in_maps = make_in_maps(inputs, T)
    res = run_bass_kernel_spmd(nc, in_maps, core_ids=list(range(N_CORES)))
    return combine(res.results, inputs, T)


# revision 26
# speedup vs baseline: 3.2760x; 1.9230x over previous
"""Trainium2 Bass kernel: multi-head causal attention (B=2, T=2048, C=1024, H=16).

Sharding: 8 cores = data parallel over B (2) x tensor parallel over head
groups (4 groups of 4 heads).  Each core computes its batch's partial
output contribution from its 4 heads through Wo rows; the host sums the 4
partials per batch (the "all-reduce") and adds the folded biases.

Device pipeline (per core, 4 heads, matmul operands bf16 / PSUM fp32):
  - Q/K/V arrive bf16 [T, C]; one DMA xbar-transpose per (tensor, 512-row
    chunk) loads them as [C, T] chunks straight into SBUF (no PE work)
  - qT/kT = W^T @ X^T + b  laid out [head_dim, T]; v kept natural [T, dv]
    with a ones column appended per head (bias bv folded on host)
  - scores computed TRANSPOSED per head PAIR: the two heads of a pair sit
    on PE row groups 0-63 / 64-127 so their K=64 score matmuls overlap in
    the array; sc/attnT/av tiles hold both heads (2-bank PSUM tiles) so
    Exp / mask / softmax-sum chains run once per pair
  - av[dv+1, q] += v_aug^T @ attnT accumulated over k blocks; the ones
    column makes row dv the softmax denominator S[q] for free; av is
    copied to SBUF once, then R=1/S (DVE), partition_broadcast + scale
    run on Pool from SBUF
  - the PE p-state ramp needs continuous work: projection / output-
    projection matmuls are interleaved into the attention stream from a
    work queue so the tensor engine never idles
"""

from collections import deque
from contextlib import ExitStack

import numpy as np
import ml_dtypes

import concourse.bass as bass
import concourse.mybir as mybir
import concourse.tile as tile
from concourse import bacc
from concourse.bass_utils import run_bass_kernel_spmd

B, T, C = 2, 2048, 1024
H, DK, DV = 16, 64, 64
N_CORES = 8
GROUPS = 4                 # head groups (tensor parallel)
HPG = H // GROUPS          # 4 heads per group
GD = HPG * DK              # 256 head dims per group
P = 128
TCH = 512                  # chunk of T for wide matmuls / query chunks

BF = mybir.dt.bfloat16
F32 = mybir.dt.float32
AX = mybir.AxisListType
AF = mybir.ActivationFunctionType

bf16 = ml_dtypes.bfloat16

# scheduling knobs; _NC_CACHE keys include these
CFG = {"tch_bufs": 4, "attnt_bufs": 2, "sc_bufs": 2, "mm_bufs": 2,
       "fin_bufs": 2, "skip_frac": 4, "pair_exp": True, "pair_mask": True,
       "pair_avsb": True, "pump": True}


def _emit(nc, tc, io, t_len, ctx):
    NT = t_len // P            # key 128-blocks
    NQC = t_len // TCH         # query 512-chunks
    NCB = C // P               # contraction chunks over C

    cpool = ctx.enter_context(tc.tile_pool(name="const", bufs=1))
    spool = ctx.enter_context(tc.tile_pool(name="stream", bufs=2))
    ppool = ctx.enter_context(tc.tile_pool(name="pers", bufs=1))
    apool = ctx.enter_context(tc.tile_pool(name="attn", bufs=2))
    pp = ctx.enter_context(tc.tile_pool(name="ps", bufs=2, space="PSUM"))

    # ---- constants / weights (single batched DMA each, SP ring) -------------
    amask2 = cpool.tile([P, 2, P], F32)  # strict LOWER tri -1e9, twice
    nc.sync.dma_start(out=amask2, in_=io["amask"][:, :].rearrange(
        "p (a b) -> p a b", a=2))
    bq_sb = cpool.tile([P, 2], F32)
    nc.sync.dma_start(out=bq_sb, in_=io["bq"][:, :])
    bk_sb = cpool.tile([P, 2], F32)
    nc.sync.dma_start(out=bk_sb, in_=io["bk"][:, :])

    wq_sb = cpool.tile([P, NCB, GD], BF)
    wk_sb = cpool.tile([P, NCB, GD], BF)
    wv_sb = cpool.tile([P, NCB, GD], BF)
    for w_sb, name in ((wq_sb, "wq"), (wk_sb, "wk"), (wv_sb, "wv")):
        nc.sync.dma_start(
            out=w_sb, in_=io[name].rearrange("(a b) c -> b a c", a=NCB))
    wo_sb = cpool.tile([P, 2, C], BF)
    nc.sync.dma_start(out=wo_sb, in_=io["wo"].rearrange("(a b) c -> b a c", a=2))

    # persistent activations
    qT_sb = ppool.tile([P, 2, t_len], BF)   # [pair head dims(128), pair, T]
    kT_sb = ppool.tile([P, 2, t_len], BF)
    # v natural [T(k), head, dv + ones col]; row DV of av becomes sum(exp)
    v_aug = ppool.tile([P, NT, HPG, DV + 2], BF)
    nc.gpsimd.memset(v_aug[:, :, :, DV:DV + 1], 1.0)
    outT_sb = ppool.tile([P, 2, t_len], BF)  # [2 heads' dv, pair, T]

    # ---- DMA-transposed loads for one t-chunk -------------------------------
    tchs = {}

    def dma_t4(t4):
        t0 = t4 * TCH
        tt = {}
        for name in ("q", "k", "v"):
            tch = spool.tile([P, NCB, TCH], BF, tag=f"tch_{name}",
                             name=f"tch_{name}", bufs=CFG["tch_bufs"] // 2)
            nc.sync.dma_start(
                out=tch, in_=io[name][t0:t0 + TCH, :], transpose=True)
            tt[name] = tch
        tchs[t4] = tt

    # ---- projection work units (one PE matmul each) -------------------------
    def proj_units(t4):
        t0 = t4 * TCH
        units = []
        for name, w_sb, bias_sb, xT_sb in (
            ("q", wq_sb, bq_sb, qT_sb),
            ("k", wk_sb, bk_sb, kT_sb),
        ):
            for pr in range(2):
                state = {}

                def unit(cb, state=state, name=name, w_sb=w_sb,
                         bias_sb=bias_sb, xT_sb=xT_sb, pr=pr):
                    if cb == 0:
                        state["ps"] = pp.tile([P, TCH], F32, tag="mm",
                                              name="ps", bufs=CFG["mm_bufs"])
                    nc.tensor.matmul(
                        state["ps"], w_sb[:, cb, pr * P:(pr + 1) * P],
                        tchs[t4][name][:, cb, :],
                        start=(cb == 0), stop=(cb == NCB - 1))
                    if cb == NCB - 1:
                        nc.vector.tensor_scalar_add(
                            xT_sb[:, pr, t0:t0 + TCH], state["ps"],
                            bias_sb[:, pr:pr + 1])
                for cb in range(NCB):
                    units.append(lambda cb=cb, u=unit: u(cb))
        for tb in range(4):
            state = {}

            def vunit(cb, state=state, tb=tb):
                if cb == 0:
                    state["ps"] = pp.tile([P, GD], F32, tag="mm",
                                          name="ps", bufs=CFG["mm_bufs"])
                nc.tensor.matmul(
                    state["ps"], tchs[t4]["v"][:, cb, tb * P:(tb + 1) * P],
                    wv_sb[:, cb, :], start=(cb == 0), stop=(cb == NCB - 1))
                if cb == NCB - 1:
                    nc.vector.tensor_copy(
                        v_aug[:, t4 * 4 + tb, :, 0:DV],
                        state["ps"].rearrange("p (h d) -> p h d", h=HPG))
            for cb in range(NCB):
                units.append(lambda cb=cb, u=vunit: u(cb))
        return units

    # ---- output projection work units (two PE matmuls each) -----------------
    def outproj_units(qc):
        units = []
        for tb in range(qc * 4, qc * 4 + 4):
            state = {}

            def unit(cc, state=state, tb=tb):
                if cc == 0:
                    state["fin"] = apool.tile([P, C], BF, tag="fin",
                                              name="fin", bufs=CFG["fin_bufs"])
                ps = pp.tile([P, TCH], F32, tag="mm", bufs=CFG["mm_bufs"])
                for pr in range(2):
                    nc.tensor.matmul(
                        ps, outT_sb[:, pr, tb * P:(tb + 1) * P],
                        wo_sb[:, pr, cc * TCH:(cc + 1) * TCH],
                        start=(pr == 0), stop=(pr == 1))
                nc.vector.tensor_copy(state["fin"][:, cc * TCH:(cc + 1) * TCH], ps)
                if cc == 1:
                    nc.sync.dma_start(out=io["out"][tb * P:(tb + 1) * P, :],
                                      in_=state["fin"])
            for cc in range(C // TCH):
                units.append(lambda cc=cc, u=unit: u(cc))
        return units

    work = deque()

    def pump(n):
        for _ in range(min(n, len(work))):
            work.popleft()()

    # ---- attention for one query chunk, pumping queued proj work ------------
    def attend_qc(qc):
        nkb = (qc + 1) * 4
        iters = 2 * nkb
        skip = max(1, iters // CFG["skip_frac"])
        it = 0
        for hp in range(2):
            pr = hp
            # diagonal blocks first: the d=0 one opens the av accumulation
            # full-width; trailing full blocks close it full-width
            order = list(range(qc * 4, nkb)) + list(range(0, qc * 4))
            attnT = apool.tile([P, nkb, 2, TCH], BF, tag="attnT",
                               name="attnT", bufs=CFG["attnt_bufs"])
            av = pp.tile([P, 2, TCH], F32, tag="av", name="av", bufs=1)

            def emit_av(idx, kb):
                d = max(kb * P - qc * TCH, 0)
                for i in (0, 1):
                    nc.tensor.matmul(
                        av[0:DV + 1, i, d:],
                        v_aug[:, kb, 2 * hp + i, 0:DV + 1],
                        attnT[:, kb, i, d:],
                        start=(idx == 0), stop=(idx == nkb - 1),
                        skip_group_check=True)

            prev = None
            for idx, kb in enumerate(order):
                d = max(kb * P - qc * TCH, 0)
                sc = pp.tile([P, 2, TCH], F32, tag="sc", name="sc",
                             bufs=CFG["sc_bufs"])
                for i in (0, 1):
                    hs = i * DK
                    nc.tensor.matmul(
                        sc[:, i, d:], kT_sb[hs:hs + DK, pr, kb * P:(kb + 1) * P],
                        qT_sb[hs:hs + DK, pr, qc * TCH + d:(qc + 1) * TCH],
                        start=True, stop=True)
                if kb * P >= qc * TCH:  # diagonal block: mask k > q
                    if CFG["pair_mask"]:
                        nc.vector.tensor_add(sc[:, :, d:d + P],
                                             sc[:, :, d:d + P], amask2)
                    else:
                        for i in (0, 1):
                            nc.vector.tensor_add(
                                sc[:, i, d:d + P], sc[:, i, d:d + P],
                                amask2[:, i, :])
                if CFG["pair_exp"]:
                    nc.scalar.activation(
                        attnT[:, kb, :, d:], sc[:, :, d:], AF.Exp, scale=0.125)
                else:
                    for i in (0, 1):
                        nc.scalar.activation(
                            attnT[:, kb, i, d:], sc[:, i, d:], AF.Exp,
                            scale=0.125)
                if prev is not None:
                    emit_av(*prev)
                prev = (idx, kb)
                it += 1
                if CFG["pump"] and it > skip and work:
                    pump(-(-len(work) // max(1, iters - it)))
            emit_av(*prev)
            # free the av banks with one copy, then normalize from SBUF
            av_sb = apool.tile([DV, 2, TCH], F32, tag="av_sb",
                               name="av_sb", bufs=2)
            if CFG["pair_avsb"]:
                nc.vector.tensor_copy(av_sb, av[0:DV, :, :])
            else:
                for i in (0, 1):
                    nc.vector.tensor_copy(av_sb[:, i, :], av[0:DV, i, :])
            # S must land on partition 0: reciprocal_approx_fast misreads
            # inputs at a nonzero base partition
            S_sb = apool.tile([1, 2, TCH], F32, tag="S_sb", name="S_sb", bufs=2)
            nc.scalar.copy(S_sb, av[DV:DV + 1, :, :])
            R32 = apool.tile([1, 2, TCH], F32, tag="R32", name="R32", bufs=2)
            nc.vector.reciprocal_approx_fast(R32, S_sb)
            Rb = apool.tile([DV, 2, TCH], F32, tag="Rb", name="Rb", bufs=2)
            nc.gpsimd.partition_broadcast(Rb, R32)
            for i in (0, 1):
                nc.vector.tensor_mul(
                    outT_sb[i * DV:(i + 1) * DV, pr, qc * TCH:(qc + 1) * TCH],
                    av_sb[0:DV, i, :], Rb[:, i, :])

    # ---- schedule -----------------------------------------------------------
    dma_t4(0)
    dma_t4(1)
    for u in proj_units(0):
        u()
    for qc in range(NQC):
        if qc + 2 < NQC:
            dma_t4(qc + 2)
        if qc + 1 < NQC:
            work.extend(proj_units(qc + 1))
        if qc >= 1:
            work.extend(outproj_units(qc - 1))
        if not CFG["pump"]:
            pump(len(work))
        attend_qc(qc)
    pump(len(work))
    for u in outproj_units(NQC - 1):
        u()


def _build(t_len=T, reps=1):
    nc = bacc.Bacc("TRN2", target_bir_lowering=False, debug=False,
                   num_devices=N_CORES)
    io = {
        "q": nc.dram_tensor("q", [t_len, C], BF, kind="ExternalInput"),
        "k": nc.dram_tensor("k", [t_len, C], BF, kind="ExternalInput"),
        "v": nc.dram_tensor("v", [t_len, C], BF, kind="ExternalInput"),
        "wq": nc.dram_tensor("wq", [C, GD], BF, kind="ExternalInput"),
        "wk": nc.dram_tensor("wk", [C, GD], BF, kind="ExternalInput"),
        "wv": nc.dram_tensor("wv", [C, GD], BF, kind="ExternalInput"),
        "wo": nc.dram_tensor("wo", [GD, C], BF, kind="ExternalInput"),
        "bq": nc.dram_tensor("bq", [P, 2], F32, kind="ExternalInput"),
        "bk": nc.dram_tensor("bk", [P, 2], F32, kind="ExternalInput"),
        "amask": nc.dram_tensor("amask", [P, 2 * P], F32, kind="ExternalInput"),
        "out": nc.dram_tensor("out", [t_len, C], BF, kind="ExternalOutput"),
    }
    with tile.TileContext(nc) as tc, ExitStack() as ctx:
        if reps == 1:
            _emit(nc, tc, io, t_len, ctx)
        else:
            hints = (mybir.EngineType.PE, mybir.EngineType.DVE,
                     mybir.EngineType.Activation, mybir.EngineType.Pool,
                     mybir.EngineType.SP)
            with tc.For_i(0, reps, 1, hint_engines=hints):
                _emit(nc, tc, io, t_len, ctx)
    nc.compile()
    return nc


_NC_CACHE = {}


def _get_nc(t_len=T, reps=1):
    key = (t_len, reps, tuple(sorted(CFG.items())))
    if key not in _NC_CACHE:
        _NC_CACHE[key] = _build(t_len, reps)
    return _NC_CACHE[key]


def _host_constants():
    amaskT = np.tril(np.full((P, P), -1e9, np.float32), -1)
    return np.concatenate([amaskT, amaskT], axis=1)


def make_in_maps(inputs, t_len=T):
    Q, K, V = inputs["Q"], inputs["K"], inputs["V"]
    Wq, bq = inputs["Wq"], inputs["bq"]
    Wk, bk = inputs["Wk"], inputs["bk"]
    Wv = inputs["Wv"]
    Wo = inputs["Wo"]
    amask2 = _host_constants()
    in_maps = []
    for core in range(N_CORES):
        b, g = divmod(core, GROUPS)
        cs = slice(g * GD, (g + 1) * GD)
        in_maps.append({
            "q": np.ascontiguousarray(Q[b, :t_len]).astype(bf16),
            "k": np.ascontiguousarray(K[b, :t_len]).astype(bf16),
            "v": np.ascontiguousarray(V[b, :t_len]).astype(bf16),
            "wq": np.ascontiguousarray(Wq[:, cs]).astype(bf16),
            "wk": np.ascontiguousarray(Wk[:, cs]).astype(bf16),
            "wv": np.ascontiguousarray(Wv[:, cs]).astype(bf16),
            "wo": np.ascontiguousarray(Wo[cs, :]).astype(bf16),
            "bq": np.ascontiguousarray(bq[cs].reshape(2, P).T).astype(np.float32),
            "bk": np.ascontiguousarray(bk[cs].reshape(2, P).T).astype(np.float32),
            "amask": amask2,
        })
    return in_maps


def combine(results, inputs, t_len=T):
    bo, bv, Wo = inputs["bo"], inputs["bv"], inputs["Wo"]
    bias = (bo.astype(np.float64) + bv.astype(np.float64) @ Wo.astype(np.float64))
    out = np.empty((B, t_len, C), np.float32)
    for b in range(B):
        acc = np.zeros((t_len, C), np.float64)
        for g in range(GROUPS):
            acc += results[b * GROUPS + g]["out"].astype(np.float64)
        out[b] = (acc + bias).astype(np.float32)
    return out


def _mask_is_causal(mask, t_len):
    mask = np.asarray(mask)
    if mask.shape != (1, 1, t_len, t_len):
        return False
    m = mask[0, 0]
    tri = np.tril(np.ones((t_len, t_len), bool))
    return (m[tri] == 0.0).all() and (m[~tri] <= -1e8).all()


def _reference_fallback(inputs):
    # generic-mask fallback (never hit with the causal reference mask)
    Q, K, V = (np.asarray(inputs[k], np.float32) for k in ("Q", "K", "V"))
    mask = np.asarray(inputs["mask"], np.float32)
    out = np.empty((B, T, C), np.float32)
    for b in range(B):
        acc = np.zeros((T, C), np.float32)
        for h in range(H):
            q = Q[b] @ inputs["Wq"][:, h * DK:(h + 1) * DK] + inputs["bq"][h * DK:(h + 1) * DK]
            k = K[b] @ inputs["Wk"][:, h * DK:(h + 1) * DK] + inputs["bk"][h * DK:(h + 1) * DK]
            v = V[b] @ inputs["Wv"][:, h * DV:(h + 1) * DV] + inputs["bv"][h * DV:(h + 1) * DV]
            m = mask[min(b, mask.shape[0] - 1), min(h, mask.shape[1] - 1)]
            s = (q @ k.T + m) / np.sqrt(DK).astype(np.float32)
            s -= s.max(-1, keepdims=True)
            e = np.exp(s)
            a = e / e.sum(-1, keepdims=True)
            acc += (a @ v) @ inputs["Wo"][h * DV:(h + 1) * DV, :]
        out[b] = acc + inputs["bo"]
    return out


def kernel(**inputs):
    inputs = {k: np.asarray(v) for k, v in inputs.items()}
    if not _mask_is_causal(inputs["mask"], T):
        return _reference_fallback(inputs)
    nc = _get_nc(T)
    in_maps = make_in_maps(inputs, T)
    res = run_bass_kernel_spmd(nc, in_maps, core_ids=list(range(N_CORES)))
    return combine(res.results, inputs, T)
